# revision 1
# baseline (speedup 1.0000x reference)
import numpy as np

# nn_GeoGATLayer: B=8, N=2048, F=256
# Strategy: data-parallel over B across the 8 NeuronCores (jax pmap);
# A_geo / distance_matrix / weights replicated. Numpy fallback for safety.
_B, _N, _F = 8, 2048, 256
_ALPHA = 0.1


def _compute_numpy(X, A_geo, distance_matrix, W_w, W_b, a1, a2, attn_b, threshold):
    X = np.asarray(X, np.float32)
    h = X @ np.asarray(W_w, np.float32).T + np.asarray(W_b, np.float32)  # (B,N,F)
    s1 = h @ np.asarray(a1, np.float32)                                  # (B,N)
    s2 = h @ np.asarray(a2, np.float32)                                  # (B,N)
    e = s1[:, :, None] + s2[:, None, :] + np.float32(attn_b[0])
    e = np.where(e >= 0, e, np.float32(_ALPHA) * e)                      # leaky relu
    D = np.array(distance_matrix, np.float32, copy=True)
    idx = np.arange(D.shape[0])
    D[idx, idx] = 1.0
    e = e + (np.float32(1.0) / (D + np.float32(1e-5)))[None, :, :]
    mask = 1.0 / (1.0 + np.exp(-10.0 * (np.asarray(A_geo, np.float32)
                                        - np.float32(threshold[0]))))
    e = e * mask[None, :, :].astype(np.float32)
    e = e - e.max(axis=-1, keepdims=True)
    ex = np.exp(e, dtype=np.float32)
    attention = ex / ex.sum(axis=-1, keepdims=True)
    h_prime = np.einsum('bnm,bmf->bnf', attention, h)
    return h_prime.astype(np.float32)


def _compute_jax(X, A_geo, distance_matrix, W_w, W_b, a1, a2, attn_b, threshold):
    import jax, jax.numpy as jnp
    devs = jax.devices()
    if len(devs) < _B:
        raise RuntimeError("need 8 devices")

    def per_core(x, A_geo, Dm, W_w, W_b, a1, a2, attn_b, threshold):
        h = jnp.einsum('nf,of->no', x, W_w) + W_b                  # (N,F)
        s1 = h @ a1
        s2 = h @ a2
        e = s1[:, None] + s2[None, :] + attn_b[0]
        e = jnp.where(e >= 0, e, _ALPHA * e)
        idx = jnp.arange(Dm.shape[0])
        D = Dm.at[idx, idx].set(1.0)
        e = e + 1.0 / (D + 1e-5)
        mask = jax.nn.sigmoid(10.0 * (A_geo - threshold[0]))
        e = e * mask
        attention = jax.nn.softmax(e, axis=-1)
        return attention @ h

    f = jax.pmap(per_core,
                 in_axes=(0, None, None, None, None, None, None, None, None),
                 devices=devs[:_B])
    out = f(jnp.asarray(X), jnp.asarray(A_geo), jnp.asarray(distance_matrix),
            jnp.asarray(W_w), jnp.asarray(W_b), jnp.asarray(a1),
            jnp.asarray(a2), jnp.asarray(attn_b), jnp.asarray(threshold))
    return np.asarray(out).astype(np.float32)


def kernel(**inputs):
    try:
        return _compute_jax(**inputs)
    except Exception:
        return _compute_numpy(**inputs)

